# revision 15
# baseline (speedup 1.0000x reference)
"""Trainium2 Bass kernel for nn_CCA_Block (cross-channel attention block).

Reference computation (per batch element, B=8 sharded one-per-core):
    q = relu(x1 @ Wq); k = relu(x1 @ Wk); v = relu(x2 @ Wv)      # 1x1 convs
    scores[c,h,g] = scale * sum_w q[h,w,c] * k[g,w,c]
    attn = softmax(scores, axis=g)
    o[h,w,c] = sum_g attn[c,h,g] * v[g,w,c]
    g = sigmoid(o @ Ws + bs)
    g = gamma * (g - mu) / sqrt(var + eps) + beta
    out = x1 + x2 * g

Sharding: data-parallel over batch across the 8 NeuronCores (batch b -> core b).

Per-core dataflow (v3):
  All HBM traffic on HWDGE (fp32 staging + wide contiguous engine casts;
  SWDGE cast-DMAs cost ~1.3us of Q7 time each and gpsimd tensor ops run
  at ~2.5ns/elem, so gpsimd stays off the hot path entirely).
  All PSUM evacuations use CONTIGUOUS free-dim APs (strided evacs measure
  ~5ns/elem vs ~0.7ns contiguous).
  QK+V interleaved per 8-pixel chunk:
    x1 staged w-major -> bf16 cast xb; x2 staged h-major -> bf16 cast
    into resident chunk x2b[i] (reused by V transposes AND the phase-G
    residual; x2 is read from HBM exactly once).
    PE tile transposes (bf16, 4 per psum bank) -> convs with the weight
    as the moving operand -> relu evacs into qk_sb [w, (h,s,c)] /
    v_sb [g, w*C+c | ones-block].
  A:  4-channel score groups per PSUM bank -> ONE wide exp per bank
      (amortizes the 352-cycle ACT fixed cost 4x) -> e tiles [g, (c,h)]
      (contiguous o-matmul lhsT) -> per-channel o matmul, rhs = strided
      v column-slice with trailing ones column (N=129, softmax
      denominator for free) -> 1/Z (DVE) -> wide contiguous normalize
      into o_sb [h, (c,w)].
  G:  oT transpose -> conv with Ws -> sigmoid -> BN affine ->
      t = x2b*g + x1c (x1 re-read bf16: staged fp32 + ACT cast,
      prefetchable during A) -> out store.
"""

import numpy as np
import ml_dtypes

B, H, W, C = 8, 128, 128, 128
N_CORES = 8
BN_EPS = 1e-3

_BUILD_CACHE: dict = {}


def _build_program(scale_val: float, delta: tuple, bias_via_dve: bool, b_zero: bool):
    """Emit + compile the per-core Bass program. All cores run the identical
    program on their own batch slice."""
    import concourse.bacc as bacc
    import concourse.mybir as mybir
    import concourse.tile as tile

    fp32 = mybir.dt.float32
    bf16 = mybir.dt.bfloat16
    AF = mybir.ActivationFunctionType
    OP = mybir.AluOpType
    delta_zero = all(d == 0.0 for d in delta)

    nc = bacc.Bacc("TRN2", target_bir_lowering=False, debug=False,
                   enable_asserts=False)

    x1_d = nc.dram_tensor("x1", [H, W, C], fp32, kind="ExternalInput")
    x2_d = nc.dram_tensor("x2", [H, W, C], fp32, kind="ExternalInput")
    wqk_d = nc.dram_tensor("wqk", [C, 2 * C], bf16, kind="ExternalInput")
    wv_d = nc.dram_tensor("wv", [C, C], bf16, kind="ExternalInput")
    ws_d = nc.dram_tensor("ws", [C, C], bf16, kind="ExternalInput")
    ident_d = nc.dram_tensor("ident", [C, C], bf16, kind="ExternalInput")
    arep_d = nc.dram_tensor("a_rep", [C, 4 * C], bf16, kind="ExternalInput")
    brep_d = nc.dram_tensor("b_rep", [C, 4 * C], bf16, kind="ExternalInput")
    bsrep_d = nc.dram_tensor("bs_rep", [C, 4 * C], fp32, kind="ExternalInput")
    out_d = nc.dram_tensor("out", [H, W, C], fp32, kind="ExternalOutput")

    x1_ap, x2_ap, out_ap = x1_d.ap(), x2_d.ap(), out_d.ap()

    with tile.TileContext(nc) as tc:
        with (
            # persistent single-buffer pools
            tc.tile_pool(name="wts", bufs=1) as p_wts,
            tc.tile_pool(name="qkv", bufs=1) as p_qkv,
            tc.tile_pool(name="obuf", bufs=1) as p_o,
            tc.tile_pool(name="x2res", bufs=16) as p_x2b,
            # streaming pools
            tc.tile_pool(name="stage", bufs=4) as p_stage,     # fp32 staging
            tc.tile_pool(name="xcast", bufs=3) as p_xcast,     # x1 bf16 chunks
            tc.tile_pool(name="xT", bufs=8) as p_xT,
            tc.tile_pool(name="eexp", bufs=4) as p_e,
            tc.tile_pool(name="rz", bufs=6) as p_rz,
            tc.tile_pool(name="gres", bufs=3) as p_g,
            tc.tile_pool(name="outt", bufs=3) as p_out,
            # psum
            tc.tile_pool(name="psA", bufs=5, space="PSUM") as ps_a,
            tc.tile_pool(name="psT", bufs=3, space="PSUM") as ps_t,
        ):
            # ---- constants ----
            wqk = p_wts.tile([C, 2 * C], bf16, tag="wqk")
            wv = p_wts.tile([C, C], bf16, tag="wv")
            ws = p_wts.tile([C, C], bf16, tag="ws")
            ident = p_wts.tile([C, C], bf16, tag="ident")
            arep = p_wts.tile([C, 4 * C], bf16, tag="arep")
            nc.sync.dma_start(wqk[:], wqk_d.ap())
            nc.sync.dma_start(wv[:], wv_d.ap())
            nc.sync.dma_start(ws[:], ws_d.ap())
            nc.sync.dma_start(ident[:], ident_d.ap())
            nc.sync.dma_start(arep[:], arep_d.ap())
            if not b_zero:
                brep = p_wts.tile([C, 4 * C], bf16, tag="brep")
                nc.sync.dma_start(brep[:], brep_d.ap())
            if bias_via_dve:
                bsrep = p_wts.tile([C, 4 * C], fp32, tag="bsrep")
                nc.sync.dma_start(bsrep[:], bsrep_d.ap())

            # persistent big buffers (free-dim layouts noted)
            qk_sb = p_qkv.tile([W, 2 * C * H], bf16, tag="qk")  # [w, h*256+s*128+c]
            # v plus a trailing ones-block: column W*C+c == 1.0 so a single
            # N=129 strided matmul computes o_unnorm and the softmax denom Z
            v_sb = p_qkv.tile([H, W * C + C], bf16, tag="v")    # [g, w*128+c]
            nc.vector.memset(v_sb[:, W * C :], 1.0)
            o_sb = p_o.tile([H, C * W], bf16, tag="o")          # [h, c*128+w]
            # x2 bf16 resident chunks: chunk i holds x2[:, 8i:8i+8, :]
            x2b = [
                p_x2b.tile([H, 8 * C], bf16, tag="x2b", name=f"x2b{i}")
                for i in range(16)
            ]

            qk4 = qk_sb[:].rearrange("w (h s c) -> w h s c", s=2, c=C)
            o3 = o_sb[:].rearrange("h (c w) -> h c w", w=W)

            def transpose4(src_fn):
                """4 PE tile-transposes into one bf16 PSUM bank.
                src_fn(j) -> [128,128] bf16 SBUF AP. Returns PSUM tile."""
                pst = ps_t.tile([C, 512], bf16, tag="pst")
                for j in range(4):
                    nc.tensor.matmul(
                        pst[:, j * C : (j + 1) * C], src_fn(j), ident[:],
                        is_transpose=True, start=(j == 0), stop=(j == 3),
                    )
                return pst

            def evac(dst, src, engine):
                if engine == "act":
                    nc.scalar.activation(dst, src, AF.Copy)
                else:
                    nc.vector.tensor_copy(dst, src)

            def relu_evac(dst, src, engine):
                if engine == "act":
                    nc.scalar.activation(dst, src, AF.Relu)
                else:
                    nc.vector.tensor_scalar(dst, src, 0.0, None, OP.max)

            # ===== Phases QK and V, interleaved per 8-pixel chunk =====
            # Software-pipelined: transposes run LAG conv-groups ahead so
            # the PE never waits on a PSUM->SBUF transpose evacuation.
            pending = []  # deferred conv-emit closures
            LAG = 2

            def drain(n):
                while len(pending) > n:
                    pending.pop(0)()

            def emit_qk_convs(xt, h0):
                def go():
                    for s2 in range(2):
                        psqk = ps_a.tile([W, 512], fp32, tag="ps", name="psqk")
                        for t in range(2):
                            nc.tensor.matmul(
                                psqk[:, t * 256 : (t + 1) * 256],
                                xt[:, (2 * s2 + t) * C : (2 * s2 + t + 1) * C],
                                wqk[:], start=(t == 0), stop=(t == 1),
                            )
                        h = h0 + 2 * s2
                        dst = qk_sb[:, h * 2 * C : (h + 2) * 2 * C]
                        relu_evac(dst, psqk[:], ("act", "dve")[s2])
                return go

            def emit_v_convs(xt, w0):
                def go():
                    psv = ps_a.tile([H, 512], fp32, tag="ps", name="psv")
                    for j in range(4):
                        nc.tensor.matmul(
                            psv[:, j * C : (j + 1) * C],
                            xt[:, j * C : (j + 1) * C], wv[:],
                            start=(j == 0), stop=(j == 3),
                        )
                    relu_evac(
                        v_sb[:, w0 * C : (w0 + 4) * C], psv[:],
                        ("act", "dve")[(w0 // 4) % 2],
                    )
                return go

            for i in range(16):
                h0 = 8 * i
                # x1 staged w-major for the QK path
                stg = p_stage.tile([W, 8 * C], fp32, tag="stg")
                nc.sync.dma_start(
                    stg[:], x1_ap[h0 : h0 + 8].rearrange("hh w c -> w hh c")
                )
                xb = p_xcast.tile([W, 8 * C], bf16, tag="xb")
                nc.scalar.activation(xb[:, : 4 * C], stg[:, : 4 * C], AF.Copy)
                nc.vector.tensor_copy(xb[:, 4 * C :], stg[:, 4 * C :])
                # x2 staged h-major; bf16 resident chunk feeds V transposes
                # now and the phase-G residual later (single HBM read)
                stg2 = p_stage.tile([H, 8 * C], fp32, tag="stg")
                nc.sync.dma_start(stg2[:], x2_ap[:, h0 : h0 + 8, :])
                nc.gpsimd.tensor_copy(x2b[i][:], stg2[:])

                for j2 in range(2):  # QK: two 4-h subgroups
                    hh = 4 * j2
                    pst = transpose4(lambda j: xb[:, (hh + j) * C : (hh + j + 1) * C])
                    xt = p_xT.tile([C, 512], bf16, tag="xT")
                    evac(xt[:], pst[:], "dve" if j2 == 0 else "act")
                    pending.append(emit_qk_convs(xt, h0 + hh))
                    drain(LAG)
                for j2 in range(2):  # V: two 4-w subgroups
                    ww = 4 * j2
                    pst = transpose4(
                        lambda j: x2b[i][:, (ww + j) * C : (ww + j + 1) * C]
                    )
                    xt = p_xT.tile([C, 512], bf16, tag="xT")
                    evac(xt[:], pst[:], "dve")
                    pending.append(emit_v_convs(xt, h0 + ww))
                    drain(LAG)
            drain(0)

            # ===== Phase A: attention over channels =====
            e_tiles = {}  # sg -> e tile [g, 4H] bf16, channels 4sg..4sg+3
            o_groups = [(c0, min(3, C - c0)) for c0 in range(0, C, 3)]
            next_og = 0

            def emit_o_group(c0, gs):
                pso = ps_a.tile([H, gs * 129], fp32, tag="ps")
                for j in range(gs):
                    c = c0 + j
                    et = e_tiles[c // 4]
                    nc.tensor.matmul(
                        pso[:, j * 129 : (j + 1) * 129],
                        et[:, (c % 4) * H : (c % 4 + 1) * H],
                        v_sb[:, c : c + W * C + 1 : C],
                        start=(j == 0), stop=(j == gs - 1),
                    )
                po = pso[:].rearrange("h (j x) -> h j x", x=129)
                rz = p_rz.tile([H, gs], fp32, tag="rz")
                nc.vector.reciprocal(rz[:], po[:, :, 128])
                if delta_zero:
                    # wide normalize: o = o_unnorm * (1/Z) with 1/Z
                    # broadcast along w; contiguous-inner dst and src
                    rzb = rz[:].unsqueeze(2).broadcast_to([H, gs, W])
                    nc.vector.tensor_tensor(
                        o3[:, c0 : c0 + gs, :], po[:, :, 0:W], rzb, OP.mult,
                    )
                else:
                    for j in range(gs):
                        c = c0 + j
                        dst = o3[:, c, :]
                        src_ap = po[:, j, 0:W]
                        if (c0 // 3) % 2 == 0:
                            nc.scalar.activation(
                                dst, src_ap, AF.Copy,
                                bias=float(delta[c]), scale=rz[:, j : j + 1],
                            )
                        else:
                            nc.vector.tensor_scalar(
                                dst, src_ap, rz[:, j : j + 1], float(delta[c]),
                                OP.mult, OP.add,
                            )

            for sg in range(32):  # 4-channel score groups
                pss = ps_a.tile([H, 4 * H], fp32, tag="ps")
                for j in range(4):
                    c = 4 * sg + j
                    nc.tensor.matmul(
                        pss[:, j * H : (j + 1) * H],
                        qk4[:, :, 1, c], qk4[:, :, 0, c],
                        start=(j == 0), stop=(j == 3),
                    )
                et = p_e.tile([H, 4 * H], bf16, tag="e4")
                nc.scalar.activation(et[:], pss[:], AF.Exp, scale=scale_val)
                e_tiles[sg] = et
                # drain o-groups whose channels are all exp'd already
                while (next_og < len(o_groups)
                       and o_groups[next_og][0] + o_groups[next_og][1] <= 4 * sg):
                    emit_o_group(*o_groups[next_og])
                    next_og += 1
            while next_og < len(o_groups):
                emit_o_group(*o_groups[next_og])
                next_og += 1

            # ===== Phase G: o -> oT -> conv -> sigmoid/BN/residual =====
            # Pipelined like QKV; x1 residual via SWDGE accumulate-DMA
            # (gpsimd Q7 is otherwise idle here).
            def emit_g_tail(xt, w0):
                def go():
                    psg = ps_a.tile([H, 512], fp32, tag="ps", name="psg")
                    for j in range(4):
                        nc.tensor.matmul(
                            psg[:, j * C : (j + 1) * C],
                            xt[:, j * H : (j + 1) * H], ws[:],
                            start=(j == 0), stop=(j == 3),
                        )
                    if bias_via_dve:
                        nc.vector.tensor_tensor(psg[:], psg[:], bsrep[:], OP.add)
                    g4 = p_g.tile([H, 512], bf16, tag="g4")
                    nc.scalar.activation(g4[:], psg[:], AF.Sigmoid)
                    nc.vector.tensor_tensor(g4[:], g4[:], arep[:], OP.mult)
                    if not b_zero:
                        nc.vector.tensor_tensor(g4[:], g4[:], brep[:], OP.add)
                    x2slice = x2b[w0 // 8][:, (w0 % 8) * C : (w0 % 8 + 4) * C]
                    t4 = p_out.tile([H, 512], fp32, tag="t4")
                    nc.vector.tensor_tensor(t4[:], x2slice, g4[:], OP.mult)
                    nc.gpsimd.dma_start(
                        t4[:], x1_ap[:, w0 : w0 + 4, :], accum_op=OP.add
                    )
                    nc.sync.dma_start(out_ap[:, w0 : w0 + 4, :], t4[:])
                return go

            for w0 in range(0, W, 4):
                pst = transpose4(lambda j: o3[:, :, w0 + j])
                xt = p_xT.tile([C, 512], bf16, tag="xT")
                evac(xt[:], pst[:], "dve" if (w0 // 4) % 2 else "act")
                pending.append(emit_g_tail(xt, w0))
                drain(LAG)
            drain(0)

    nc.compile()
    return nc


def _prepare(inputs):
    """Host-side prep: derived small tensors + baked scalars."""
    x1 = np.ascontiguousarray(np.asarray(inputs["x1"], dtype=np.float32))
    x2 = np.ascontiguousarray(np.asarray(inputs["x2"], dtype=np.float32))
    Wq = np.asarray(inputs["Wq"], dtype=np.float32)
    Wk = np.asarray(inputs["Wk"], dtype=np.float32)
    Wv = np.asarray(inputs["Wv"], dtype=np.float32)
    Ws = np.asarray(inputs["Ws"], dtype=np.float32)
    bs = np.asarray(inputs["bs"], dtype=np.float32)
    scale = float(np.asarray(inputs["scale"]).reshape(-1)[0])
    gamma = np.asarray(inputs["gamma"], dtype=np.float32)
    beta = np.asarray(inputs["beta"], dtype=np.float32)
    mu = np.asarray(inputs["mu"], dtype=np.float32)
    var = np.asarray(inputs["var"], dtype=np.float32)

    a = gamma / np.sqrt(var + BN_EPS)
    b = beta - mu * a
    b_zero = bool(np.all(b == 0.0))

    # fold the sigmoid bias bs into o:  o' = o + delta with Ws^T delta = bs
    bias_via_dve = False
    delta = np.zeros(C, dtype=np.float64)
    if np.any(bs != 0.0):
        try:
            delta = np.linalg.solve(Ws.astype(np.float64).T, bs.astype(np.float64))
            resid = np.abs(Ws.T @ delta.astype(np.float32) - bs).max()
            if not np.isfinite(delta).all() or resid > 1e-5 * (1 + np.abs(bs).max()):
                raise np.linalg.LinAlgError("bad solve")
        except np.linalg.LinAlgError:
            delta = np.zeros(C, dtype=np.float64)
            bias_via_dve = True

    bf = ml_dtypes.bfloat16
    consts = {
        "wqk": np.concatenate([Wq, Wk], axis=1).astype(bf),
        "wv": Wv.astype(bf),
        "ws": Ws.astype(bf),
        "ident": np.eye(C, dtype=bf),
        "a_rep": np.tile(a, (C, 4)).astype(bf),
        "b_rep": np.tile(b, (C, 4)).astype(bf),
        "bs_rep": np.tile(bs, (C, 4)).astype(np.float32),
    }
    key = (scale, tuple(np.round(delta, 12)), bias_via_dve, b_zero)
    return x1, x2, consts, key, scale, delta, bias_via_dve, b_zero


def _get_nc(key, scale, delta, bias_via_dve, b_zero):
    if key not in _BUILD_CACHE:
        _BUILD_CACHE[key] = _build_program(scale, delta, bias_via_dve, b_zero)
    return _BUILD_CACHE[key]


def run(inputs, trace: bool = False):
    from concourse.bass_utils import run_bass_kernel_spmd

    x1, x2, consts, key, scale, delta, bias_via_dve, b_zero = _prepare(inputs)
    nc = _get_nc(key, scale, delta, bias_via_dve, b_zero)

    in_maps = []
    for core in range(N_CORES):
        m = dict(consts)
        m["x1"] = x1[core]
        m["x2"] = x2[core]
        in_maps.append(m)

    res = run_bass_kernel_spmd(
        nc, in_maps, core_ids=list(range(N_CORES)), trace=trace
    )
    out = np.stack([res.results[i]["out"] for i in range(N_CORES)], axis=0)
    return out.astype(np.float32), res


def kernel(**inputs) -> np.ndarray:
    out, _ = run(inputs, trace=False)
    return out


# revision 21
# speedup vs baseline: 1.0244x; 1.0244x over previous
"""Trainium2 Bass kernel for nn_CCA_Block (cross-channel attention block).

Reference computation (per batch element, B=8 sharded one-per-core):
    q = relu(x1 @ Wq); k = relu(x1 @ Wk); v = relu(x2 @ Wv)      # 1x1 convs
    scores[c,h,g] = scale * sum_w q[h,w,c] * k[g,w,c]
    attn = softmax(scores, axis=g)
    o[h,w,c] = sum_g attn[c,h,g] * v[g,w,c]
    g = sigmoid(o @ Ws + bs)
    g = gamma * (g - mu) / sqrt(var + eps) + beta
    out = x1 + x2 * g

Sharding: data-parallel over batch across the 8 NeuronCores (batch b -> core b).

Per-core dataflow (v3):
  All HBM traffic on HWDGE (fp32 staging + wide contiguous engine casts;
  SWDGE cast-DMAs cost ~1.3us of Q7 time each and gpsimd tensor ops run
  at ~2.5ns/elem, so gpsimd stays off the hot path entirely).
  All PSUM evacuations use CONTIGUOUS free-dim APs (strided evacs measure
  ~5ns/elem vs ~0.7ns contiguous).
  QK+V interleaved per 8-pixel chunk:
    x1 staged w-major -> bf16 cast xb; x2 staged h-major -> bf16 cast
    into resident chunk x2b[i] (reused by V transposes AND the phase-G
    residual; x2 is read from HBM exactly once).
    PE tile transposes (bf16, 4 per psum bank) -> convs with the weight
    as the moving operand -> relu evacs into qk_sb [w, (h,s,c)] /
    v_sb [g, w*C+c | ones-block].
  A:  4-channel score groups per PSUM bank -> ONE wide exp per bank
      (amortizes the 352-cycle ACT fixed cost 4x) -> e tiles [g, (c,h)]
      (contiguous o-matmul lhsT) -> per-channel o matmul, rhs = strided
      v column-slice with trailing ones column (N=129, softmax
      denominator for free) -> 1/Z (DVE) -> wide contiguous normalize
      into o_sb [h, (c,w)].
  G:  oT transpose -> conv with Ws -> sigmoid -> BN affine ->
      t = x2b*g + x1c (x1 re-read bf16: staged fp32 + ACT cast,
      prefetchable during A) -> out store.
"""

import numpy as np
import ml_dtypes

B, H, W, C = 8, 128, 128, 128
N_CORES = 8
BN_EPS = 1e-3

_BUILD_CACHE: dict = {}


def _build_program(scale_val: float, delta: tuple, bias_via_dve: bool, b_zero: bool):
    """Emit + compile the per-core Bass program. All cores run the identical
    program on their own batch slice."""
    import concourse.bacc as bacc
    import concourse.mybir as mybir
    import concourse.tile as tile

    fp32 = mybir.dt.float32
    bf16 = mybir.dt.bfloat16
    AF = mybir.ActivationFunctionType
    OP = mybir.AluOpType
    delta_zero = all(d == 0.0 for d in delta)

    nc = bacc.Bacc("TRN2", target_bir_lowering=False, debug=False,
                   enable_asserts=False)

    x1_d = nc.dram_tensor("x1", [H, W, C], fp32, kind="ExternalInput")
    x2_d = nc.dram_tensor("x2", [H, W, C], fp32, kind="ExternalInput")
    wqk_d = nc.dram_tensor("wqk", [C, 2 * C], bf16, kind="ExternalInput")
    wv_d = nc.dram_tensor("wv", [C, C], bf16, kind="ExternalInput")
    ws_d = nc.dram_tensor("ws", [C, C], bf16, kind="ExternalInput")
    ident_d = nc.dram_tensor("ident", [C, C], bf16, kind="ExternalInput")
    arep_d = nc.dram_tensor("a_rep", [C, 4 * C], bf16, kind="ExternalInput")
    brep_d = nc.dram_tensor("b_rep", [C, 4 * C], bf16, kind="ExternalInput")
    bsrep_d = nc.dram_tensor("bs_rep", [C, 4 * C], fp32, kind="ExternalInput")
    out_d = nc.dram_tensor("out", [H, W, C], fp32, kind="ExternalOutput")

    x1_ap, x2_ap, out_ap = x1_d.ap(), x2_d.ap(), out_d.ap()

    with tile.TileContext(nc) as tc:
        with (
            # persistent single-buffer pools
            tc.tile_pool(name="wts", bufs=1) as p_wts,
            tc.tile_pool(name="qkv", bufs=1) as p_qkv,
            tc.tile_pool(name="obuf", bufs=1) as p_o,
            tc.tile_pool(name="x2res", bufs=16) as p_x2b,
            # streaming pools
            tc.tile_pool(name="stage", bufs=4) as p_stage,     # fp32 staging
            tc.tile_pool(name="xcast", bufs=3) as p_xcast,     # x1 bf16 chunks
            tc.tile_pool(name="xT", bufs=8) as p_xT,
            tc.tile_pool(name="eexp", bufs=4) as p_e,
            tc.tile_pool(name="rz", bufs=6) as p_rz,
            tc.tile_pool(name="gres", bufs=3) as p_g,
            tc.tile_pool(name="outt", bufs=2) as p_out,
            # psum
            tc.tile_pool(name="psA", bufs=5, space="PSUM") as ps_a,
            tc.tile_pool(name="psT", bufs=3, space="PSUM") as ps_t,
        ):
            # ---- constants ----
            wqk = p_wts.tile([C, 2 * C], bf16, tag="wqk")
            wv = p_wts.tile([C, C], bf16, tag="wv")
            ws = p_wts.tile([C, C], bf16, tag="ws")
            ident = p_wts.tile([C, C], bf16, tag="ident")
            arep = p_wts.tile([C, 4 * C], bf16, tag="arep")
            nc.sync.dma_start(wqk[:], wqk_d.ap())
            nc.sync.dma_start(wv[:], wv_d.ap())
            nc.sync.dma_start(ws[:], ws_d.ap())
            nc.sync.dma_start(ident[:], ident_d.ap())
            nc.sync.dma_start(arep[:], arep_d.ap())
            if not b_zero:
                brep = p_wts.tile([C, 4 * C], bf16, tag="brep")
                nc.sync.dma_start(brep[:], brep_d.ap())
            if bias_via_dve:
                bsrep = p_wts.tile([C, 4 * C], fp32, tag="bsrep")
                nc.sync.dma_start(bsrep[:], bsrep_d.ap())

            # persistent big buffers (free-dim layouts noted)
            qk_sb = p_qkv.tile([W, 2 * C * H], bf16, tag="qk")  # [w, h*256+s*128+c]
            # v plus a trailing ones-block: column W*C+c == 1.0 so a single
            # N=129 strided matmul computes o_unnorm and the softmax denom Z
            v_sb = p_qkv.tile([H, W * C + C], bf16, tag="v")    # [g, w*128+c]
            nc.vector.memset(v_sb[:, W * C :], 1.0)
            o_sb = p_o.tile([H, W * C], bf16, tag="o")          # [h, w*128+c]
            # x2 bf16 resident chunks: chunk i holds x2[:, 8i:8i+8, :]
            x2b = [
                p_x2b.tile([H, 8 * C], bf16, tag="x2b", name=f"x2b{i}")
                for i in range(16)
            ]

            qk4 = qk_sb[:].rearrange("w (h s c) -> w h s c", s=2, c=C)
            o3 = o_sb[:].rearrange("h (w c) -> h w c", c=C)

            def transpose4(src_fn):
                """4 PE tile-transposes into one bf16 PSUM bank.
                src_fn(j) -> [128,128] bf16 SBUF AP. Returns PSUM tile."""
                pst = ps_t.tile([C, 512], bf16, tag="pst")
                for j in range(4):
                    nc.tensor.matmul(
                        pst[:, j * C : (j + 1) * C], src_fn(j), ident[:],
                        is_transpose=True, start=(j == 0), stop=(j == 3),
                    )
                return pst

            def evac(dst, src, engine):
                if engine == "act":
                    nc.scalar.activation(dst, src, AF.Copy)
                else:
                    nc.vector.tensor_copy(dst, src)

            def relu_evac(dst, src, engine):
                if engine == "act":
                    nc.scalar.activation(dst, src, AF.Relu)
                else:
                    nc.vector.tensor_scalar(dst, src, 0.0, None, OP.max)

            # ===== Phases QK and V, interleaved per 8-pixel chunk =====
            # Software-pipelined: transposes run LAG conv-groups ahead so
            # the PE never waits on a PSUM->SBUF transpose evacuation.
            pending = []  # deferred conv-emit closures
            LAG = 2

            def drain(n):
                while len(pending) > n:
                    pending.pop(0)()

            def emit_qk_convs(xt, h0):
                def go():
                    for s2 in range(2):
                        psqk = ps_a.tile([W, 512], fp32, tag="ps", name="psqk")
                        for t in range(2):
                            nc.tensor.matmul(
                                psqk[:, t * 256 : (t + 1) * 256],
                                xt[:, (2 * s2 + t) * C : (2 * s2 + t + 1) * C],
                                wqk[:], start=(t == 0), stop=(t == 1),
                            )
                        h = h0 + 2 * s2
                        dst = qk_sb[:, h * 2 * C : (h + 2) * 2 * C]
                        relu_evac(dst, psqk[:], ("act", "dve")[s2])
                return go

            def emit_v_convs(xt, w0):
                def go():
                    psv = ps_a.tile([H, 512], fp32, tag="ps", name="psv")
                    for j in range(4):
                        nc.tensor.matmul(
                            psv[:, j * C : (j + 1) * C],
                            xt[:, j * C : (j + 1) * C], wv[:],
                            start=(j == 0), stop=(j == 3),
                        )
                    relu_evac(
                        v_sb[:, w0 * C : (w0 + 4) * C], psv[:],
                        ("act", "dve")[(w0 // 4) % 2],
                    )
                return go

            xb_tiles = {}

            def emit_chunk_loads(i):
                """Stage + cast chunk i (emitted one iteration ahead so the
                casts overlap the previous chunk's transposes/convs)."""
                h0 = 8 * i
                # x1 staged w-major for the QK path
                stg = p_stage.tile([W, 8 * C], fp32, tag="stg", name="stg1")
                nc.sync.dma_start(
                    stg[:], x1_ap[h0 : h0 + 8].rearrange("hh w c -> w hh c")
                )
                xb = p_xcast.tile([W, 8 * C], bf16, tag="xb")
                nc.scalar.activation(xb[:, : 4 * C], stg[:, : 4 * C], AF.Copy)
                nc.vector.tensor_copy(xb[:, 4 * C :], stg[:, 4 * C :])
                xb_tiles[i] = xb
                # x2 staged h-major; bf16 resident chunk feeds V transposes
                # and the phase-G residual later (single HBM read)
                stg2 = p_stage.tile([H, 8 * C], fp32, tag="stg", name="stg2")
                nc.sync.dma_start(stg2[:], x2_ap[:, h0 : h0 + 8, :])
                nc.gpsimd.tensor_copy(x2b[i][:], stg2[:])

            emit_chunk_loads(0)
            for i in range(16):
                h0 = 8 * i
                if i + 1 < 16:
                    emit_chunk_loads(i + 1)
                xb = xb_tiles.pop(i)

                for j2 in range(2):  # QK: two 4-h subgroups
                    hh = 4 * j2
                    pst = transpose4(lambda j: xb[:, (hh + j) * C : (hh + j + 1) * C])
                    xt = p_xT.tile([C, 512], bf16, tag="xT")
                    evac(xt[:], pst[:], "dve" if j2 == 0 else "act")
                    pending.append(emit_qk_convs(xt, h0 + hh))
                    drain(LAG)
                for j2 in range(2):  # V: two 4-w subgroups
                    ww = 4 * j2
                    pst = transpose4(
                        lambda j: x2b[i][:, (ww + j) * C : (ww + j + 1) * C]
                    )
                    xt = p_xT.tile([C, 512], bf16, tag="xT")
                    evac(xt[:], pst[:], "dve")
                    pending.append(emit_v_convs(xt, h0 + ww))
                    drain(LAG)
            drain(0)

            # ===== Phase A: attention over channels =====
            e_tiles = {}  # sg -> e tile [g, 4H] bf16, channels 4sg..4sg+3
            o_groups = [(c0, min(3, C - c0)) for c0 in range(0, C, 3)]
            next_og = 0

            def emit_o_group(c0, gs):
                pso = ps_a.tile([H, gs * 129], fp32, tag="ps")
                for j in range(gs):
                    c = c0 + j
                    et = e_tiles[c // 4]
                    nc.tensor.matmul(
                        pso[:, j * 129 : (j + 1) * 129],
                        et[:, (c % 4) * H : (c % 4 + 1) * H],
                        v_sb[:, c : c + W * C + 1 : C],
                        start=(j == 0), stop=(j == gs - 1),
                    )
                po = pso[:].rearrange("h (j x) -> h j x", x=129)
                rz = p_rz.tile([H, gs], fp32, tag="rz")
                nc.vector.reciprocal(rz[:], po[:, :, 128])
                if delta_zero:
                    # wide normalize: o = o_unnorm * (1/Z) with 1/Z
                    # broadcast along w; dst is o_sb pixel-major [h,(w,c)]
                    pox = pso[:].rearrange("h (j x) -> h x j", x=129)
                    rzb = rz[:].unsqueeze(1).broadcast_to([H, W, gs])
                    nc.vector.tensor_tensor(
                        o3[:, :, c0 : c0 + gs], pox[:, 0:W, :], rzb, OP.mult,
                    )
                else:
                    for j in range(gs):
                        c = c0 + j
                        dst = o3[:, :, c]
                        src_ap = po[:, j, 0:W]
                        if (c0 // 3) % 2 == 0:
                            nc.scalar.activation(
                                dst, src_ap, AF.Copy,
                                bias=float(delta[c]), scale=rz[:, j : j + 1],
                            )
                        else:
                            nc.vector.tensor_scalar(
                                dst, src_ap, rz[:, j : j + 1], float(delta[c]),
                                OP.mult, OP.add,
                            )

            for sg in range(32):  # 4-channel score groups
                pss = ps_a.tile([H, 4 * H], fp32, tag="ps")
                for j in range(4):
                    c = 4 * sg + j
                    nc.tensor.matmul(
                        pss[:, j * H : (j + 1) * H],
                        qk4[:, :, 1, c], qk4[:, :, 0, c],
                        start=(j == 0), stop=(j == 3),
                    )
                et = p_e.tile([H, 4 * H], bf16, tag="e4")
                nc.scalar.activation(et[:], pss[:], AF.Exp, scale=scale_val)
                e_tiles[sg] = et
                # drain o-groups whose channels are all exp'd already
                while (next_og < len(o_groups)
                       and o_groups[next_og][0] + o_groups[next_og][1] <= 4 * sg):
                    emit_o_group(*o_groups[next_og])
                    next_og += 1
            while next_og < len(o_groups):
                emit_o_group(*o_groups[next_og])
                next_og += 1

            # ===== Phase G: o -> oT -> conv -> sigmoid/BN/residual =====
            # Pipelined like QKV; x1 residual via SWDGE accumulate-DMA
            # (gpsimd Q7 is otherwise idle here).
            t8_tiles = {}

            def emit_g_tail(xt, w0):
                def go():
                    psg = ps_a.tile([H, 512], fp32, tag="ps", name="psg")
                    for j in range(4):
                        nc.tensor.matmul(
                            psg[:, j * C : (j + 1) * C],
                            xt[:, j * H : (j + 1) * H], ws[:],
                            start=(j == 0), stop=(j == 3),
                        )
                    if bias_via_dve:
                        nc.vector.tensor_tensor(psg[:], psg[:], bsrep[:], OP.add)
                    g4 = p_g.tile([H, 512], bf16, tag="g4")
                    nc.scalar.activation(g4[:], psg[:], AF.Sigmoid)
                    nc.vector.tensor_tensor(g4[:], g4[:], arep[:], OP.mult)
                    if not b_zero:
                        nc.vector.tensor_tensor(g4[:], g4[:], brep[:], OP.add)
                    x2slice = x2b[w0 // 8][:, (w0 % 8) * C : (w0 % 8 + 4) * C]
                    # t8 spans 8 w's; residual accumulate + store done 8-wide
                    # to halve the SWDGE Q7 descriptor-gen cost
                    half = w0 % 8
                    if half == 0:
                        t8_tiles[w0 // 8] = p_out.tile(
                            [H, 1024], fp32, tag="t8", name="t8"
                        )
                    t8 = t8_tiles[w0 // 8]
                    nc.vector.tensor_tensor(
                        t8[:, half * C : (half + 4) * C], x2slice, g4[:], OP.mult
                    )
                    if half == 4:
                        w8 = (w0 // 8) * 8
                        nc.gpsimd.dma_start(
                            t8[:], x1_ap[:, w8 : w8 + 8, :], accum_op=OP.add
                        )
                        nc.sync.dma_start(out_ap[:, w8 : w8 + 8, :], t8[:])
                return go

            for w0 in range(0, W, 4):
                pst = transpose4(
                    lambda j: o_sb[:, (w0 + j) * C : (w0 + j + 1) * C]
                )
                xt = p_xT.tile([C, 512], bf16, tag="xT")
                evac(xt[:], pst[:], "dve" if (w0 // 4) % 2 else "act")
                pending.append(emit_g_tail(xt, w0))
                drain(LAG)
            drain(0)

    nc.compile()
    return nc


def _prepare(inputs):
    """Host-side prep: derived small tensors + baked scalars."""
    x1 = np.ascontiguousarray(np.asarray(inputs["x1"], dtype=np.float32))
    x2 = np.ascontiguousarray(np.asarray(inputs["x2"], dtype=np.float32))
    Wq = np.asarray(inputs["Wq"], dtype=np.float32)
    Wk = np.asarray(inputs["Wk"], dtype=np.float32)
    Wv = np.asarray(inputs["Wv"], dtype=np.float32)
    Ws = np.asarray(inputs["Ws"], dtype=np.float32)
    bs = np.asarray(inputs["bs"], dtype=np.float32)
    scale = float(np.asarray(inputs["scale"]).reshape(-1)[0])
    gamma = np.asarray(inputs["gamma"], dtype=np.float32)
    beta = np.asarray(inputs["beta"], dtype=np.float32)
    mu = np.asarray(inputs["mu"], dtype=np.float32)
    var = np.asarray(inputs["var"], dtype=np.float32)

    a = gamma / np.sqrt(var + BN_EPS)
    b = beta - mu * a
    b_zero = bool(np.all(b == 0.0))

    # fold the sigmoid bias bs into o:  o' = o + delta with Ws^T delta = bs
    bias_via_dve = False
    delta = np.zeros(C, dtype=np.float64)
    if np.any(bs != 0.0):
        try:
            delta = np.linalg.solve(Ws.astype(np.float64).T, bs.astype(np.float64))
            resid = np.abs(Ws.T @ delta.astype(np.float32) - bs).max()
            if not np.isfinite(delta).all() or resid > 1e-5 * (1 + np.abs(bs).max()):
                raise np.linalg.LinAlgError("bad solve")
        except np.linalg.LinAlgError:
            delta = np.zeros(C, dtype=np.float64)
            bias_via_dve = True

    bf = ml_dtypes.bfloat16
    consts = {
        "wqk": np.concatenate([Wq, Wk], axis=1).astype(bf),
        "wv": Wv.astype(bf),
        "ws": Ws.astype(bf),
        "ident": np.eye(C, dtype=bf),
        "a_rep": np.tile(a, (C, 4)).astype(bf),
        "b_rep": np.tile(b, (C, 4)).astype(bf),
        "bs_rep": np.tile(bs, (C, 4)).astype(np.float32),
    }
    key = (scale, tuple(np.round(delta, 12)), bias_via_dve, b_zero)
    return x1, x2, consts, key, scale, delta, bias_via_dve, b_zero


def _get_nc(key, scale, delta, bias_via_dve, b_zero):
    if key not in _BUILD_CACHE:
        _BUILD_CACHE[key] = _build_program(scale, delta, bias_via_dve, b_zero)
    return _BUILD_CACHE[key]


def run(inputs, trace: bool = False):
    from concourse.bass_utils import run_bass_kernel_spmd

    x1, x2, consts, key, scale, delta, bias_via_dve, b_zero = _prepare(inputs)
    nc = _get_nc(key, scale, delta, bias_via_dve, b_zero)

    in_maps = []
    for core in range(N_CORES):
        m = dict(consts)
        m["x1"] = x1[core]
        m["x2"] = x2[core]
        in_maps.append(m)

    res = run_bass_kernel_spmd(
        nc, in_maps, core_ids=list(range(N_CORES)), trace=trace
    )
    out = np.stack([res.results[i]["out"] for i in range(N_CORES)], axis=0)
    return out.astype(np.float32), res


def kernel(**inputs) -> np.ndarray:
    out, _ = run(inputs, trace=False)
    return out


# revision 26
# speedup vs baseline: 1.2223x; 1.1932x over previous
"""Trainium2 Bass kernel for nn_CCA_Block (cross-channel attention block).

Reference computation (per batch element, B=8 sharded one-per-core):
    q = relu(x1 @ Wq); k = relu(x1 @ Wk); v = relu(x2 @ Wv)      # 1x1 convs
    scores[c,h,g] = scale * sum_w q[h,w,c] * k[g,w,c]
    attn = softmax(scores, axis=g)
    o[h,w,c] = sum_g attn[c,h,g] * v[g,w,c]
    g = sigmoid(o @ Ws + bs)
    g = gamma * (g - mu) / sqrt(var + eps) + beta
    out = x1 + x2 * g

Sharding: data-parallel over batch across the 8 NeuronCores (batch b -> core b).

Per-core dataflow (v3):
  All HBM traffic on HWDGE (fp32 staging + wide contiguous engine casts;
  SWDGE cast-DMAs cost ~1.3us of Q7 time each and gpsimd tensor ops run
  at ~2.5ns/elem, so gpsimd stays off the hot path entirely).
  All PSUM evacuations use CONTIGUOUS free-dim APs (strided evacs measure
  ~5ns/elem vs ~0.7ns contiguous).
  QK+V interleaved per 8-pixel chunk:
    x1 staged w-major -> bf16 cast xb; x2 staged h-major -> bf16 cast
    into resident chunk x2b[i] (reused by V transposes AND the phase-G
    residual; x2 is read from HBM exactly once).
    PE tile transposes (bf16, 4 per psum bank) -> convs with the weight
    as the moving operand -> relu evacs into qk_sb [w, (h,s,c)] /
    v_sb [g, w*C+c | ones-block].
  A:  4-channel score groups per PSUM bank -> ONE wide exp per bank
      (amortizes the 352-cycle ACT fixed cost 4x) -> e tiles [g, (c,h)]
      (contiguous o-matmul lhsT) -> per-channel o matmul, rhs = strided
      v column-slice with trailing ones column (N=129, softmax
      denominator for free) -> 1/Z (DVE) -> wide contiguous normalize
      into o_sb [h, (c,w)].
  G:  oT transpose -> conv with Ws -> sigmoid -> BN affine ->
      t = x2b*g + x1c (x1 re-read bf16: staged fp32 + ACT cast,
      prefetchable during A) -> out store.
"""

import numpy as np
import ml_dtypes

B, H, W, C = 8, 128, 128, 128
N_CORES = 8
BN_EPS = 1e-3

_BUILD_CACHE: dict = {}


def _build_program(scale_val: float, delta: tuple, bias_via_dve: bool, b_zero: bool):
    """Emit + compile the per-core Bass program. All cores run the identical
    program on their own batch slice."""
    import concourse.bacc as bacc
    import concourse.mybir as mybir
    import concourse.tile as tile

    fp32 = mybir.dt.float32
    bf16 = mybir.dt.bfloat16
    AF = mybir.ActivationFunctionType
    OP = mybir.AluOpType
    delta_zero = all(d == 0.0 for d in delta)

    nc = bacc.Bacc("TRN2", target_bir_lowering=False, debug=False,
                   enable_asserts=False)

    x1_d = nc.dram_tensor("x1", [H, W, C], fp32, kind="ExternalInput")
    x2_d = nc.dram_tensor("x2", [H, W, C], fp32, kind="ExternalInput")
    wqk_d = nc.dram_tensor("wqk", [C, 2 * C], bf16, kind="ExternalInput")
    wv_d = nc.dram_tensor("wv", [C, C], bf16, kind="ExternalInput")
    ws_d = nc.dram_tensor("ws", [C, C], bf16, kind="ExternalInput")
    ident_d = nc.dram_tensor("ident", [C, C], bf16, kind="ExternalInput")
    arep_d = nc.dram_tensor("a_rep", [C, 4 * C], bf16, kind="ExternalInput")
    brep_d = nc.dram_tensor("b_rep", [C, 4 * C], bf16, kind="ExternalInput")
    bsrep_d = nc.dram_tensor("bs_rep", [C, 4 * C], fp32, kind="ExternalInput")
    out_d = nc.dram_tensor("out", [H, W, C], fp32, kind="ExternalOutput")

    x1_ap, x2_ap, out_ap = x1_d.ap(), x2_d.ap(), out_d.ap()

    with tile.TileContext(nc) as tc:
        with (
            # persistent single-buffer pools
            tc.tile_pool(name="wts", bufs=1) as p_wts,
            tc.tile_pool(name="qkv", bufs=1) as p_qkv,
            tc.tile_pool(name="obuf", bufs=1) as p_o,
            tc.tile_pool(name="x2res", bufs=16) as p_x2b,
            # streaming pools
            tc.tile_pool(name="xcast", bufs=3) as p_xcast,     # x1 bf16 chunks
            tc.tile_pool(name="xT", bufs=4) as p_xT,
            tc.tile_pool(name="eexp", bufs=4) as p_e,
            tc.tile_pool(name="rz", bufs=6) as p_rz,
            tc.tile_pool(name="gres", bufs=3) as p_g,
            tc.tile_pool(name="outt", bufs=4) as p_out,
            # psum
            tc.tile_pool(name="psA", bufs=5, space="PSUM") as ps_a,
            tc.tile_pool(name="psT", bufs=3, space="PSUM") as ps_t,
        ):
            # ---- constants ----
            wqk = p_wts.tile([C, 2 * C], bf16, tag="wqk")
            wv = p_wts.tile([C, C], bf16, tag="wv")
            ws = p_wts.tile([C, C], bf16, tag="ws")
            ident = p_wts.tile([C, C], bf16, tag="ident")
            arep = p_wts.tile([C, 4 * C], bf16, tag="arep")
            nc.sync.dma_start(wqk[:], wqk_d.ap())
            nc.sync.dma_start(wv[:], wv_d.ap())
            nc.sync.dma_start(ws[:], ws_d.ap())
            nc.sync.dma_start(ident[:], ident_d.ap())
            nc.sync.dma_start(arep[:], arep_d.ap())
            if not b_zero:
                brep = p_wts.tile([C, 4 * C], bf16, tag="brep")
                nc.sync.dma_start(brep[:], brep_d.ap())
            if bias_via_dve:
                bsrep = p_wts.tile([C, 4 * C], fp32, tag="bsrep")
                nc.sync.dma_start(bsrep[:], bsrep_d.ap())

            # persistent big buffers (free-dim layouts noted)
            qk_sb = p_qkv.tile([W, 2 * C * H], bf16, tag="qk")  # [w, h*256+s*128+c]
            # v plus a trailing ones-block: column W*C+c == 1.0 so a single
            # N=129 strided matmul computes o_unnorm and the softmax denom Z
            v_sb = p_qkv.tile([H, W * C + C], bf16, tag="v")    # [g, w*128+c]
            nc.vector.memset(v_sb[:, W * C :], 1.0)
            o_sb = p_o.tile([H, W * C], bf16, tag="o")          # [h, w*128+c]
            # x2 bf16 resident chunks: chunk i holds x2[:, 8i:8i+8, :]
            x2b = [
                p_x2b.tile([H, 8 * C], bf16, tag="x2b", name=f"x2b{i}")
                for i in range(16)
            ]

            qk4 = qk_sb[:].rearrange("w (h s c) -> w h s c", s=2, c=C)
            o3 = o_sb[:].rearrange("h (w c) -> h w c", c=C)

            def transpose8(src_fn):
                """8 PE tile-transposes into one full bf16 PSUM bank so the
                evacuation is a single wide op (amortizes fixed cost).
                src_fn(j) -> [128,128] bf16 SBUF AP. Returns PSUM tile."""
                pst = ps_t.tile([C, 1024], bf16, tag="pst")
                for j in range(8):
                    nc.tensor.matmul(
                        pst[:, j * C : (j + 1) * C], src_fn(j), ident[:],
                        is_transpose=True, start=(j == 0), stop=(j == 7),
                    )
                return pst

            def evac(dst, src, engine):
                if engine == "act":
                    nc.scalar.activation(dst, src, AF.Copy)
                else:
                    nc.vector.tensor_copy(dst, src)

            def relu_evac(dst, src, engine):
                if engine == "act":
                    nc.scalar.activation(dst, src, AF.Relu)
                else:
                    nc.vector.tensor_scalar(dst, src, 0.0, None, OP.max)

            # ===== Phases QK and V, interleaved per 8-pixel chunk =====
            # Software-pipelined: transposes run LAG conv-groups ahead so
            # the PE never waits on a PSUM->SBUF transpose evacuation.
            pending = []  # deferred conv-emit closures
            LAG = 2

            def drain(n):
                while len(pending) > n:
                    pending.pop(0)()

            def emit_qk_convs(xt, h0, off):
                def go():
                    for s2 in range(2):
                        psqk = ps_a.tile([W, 512], fp32, tag="ps", name="psqk")
                        for t in range(2):
                            u = off + 2 * s2 + t
                            nc.tensor.matmul(
                                psqk[:, t * 256 : (t + 1) * 256],
                                xt[:, u * C : (u + 1) * C],
                                wqk[:], start=(t == 0), stop=(t == 1),
                            )
                        h = h0 + off + 2 * s2
                        dst = qk_sb[:, h * 2 * C : (h + 2) * 2 * C]
                        relu_evac(dst, psqk[:], ("act", "dve")[s2])
                return go

            def emit_v_convs(xt, h0, off):
                def go():
                    w0 = h0 + off
                    psv = ps_a.tile([H, 512], fp32, tag="ps", name="psv")
                    for j in range(4):
                        nc.tensor.matmul(
                            psv[:, j * C : (j + 1) * C],
                            xt[:, (off + j) * C : (off + j + 1) * C], wv[:],
                            start=(j == 0), stop=(j == 3),
                        )
                    relu_evac(
                        v_sb[:, w0 * C : (w0 + 4) * C], psv[:],
                        ("act", "dve")[(w0 // 4) % 2],
                    )
                return go

            xb_tiles = {}

            def emit_chunk_loads(i):
                """SWDGE cast-DMA loads for chunk i (emitted one iteration
                ahead; ~1.3us of otherwise-idle Q7 time each, zero ACT/DVE)."""
                h0 = 8 * i
                xb = p_xcast.tile([W, 8 * C], bf16, tag="xb")
                nc.gpsimd.dma_start(
                    xb[:], x1_ap[h0 : h0 + 8].rearrange("hh w c -> w hh c")
                )
                xb_tiles[i] = xb
                # x2 bf16 resident chunk feeds V transposes now and the
                # phase-G residual later (single HBM read)
                nc.gpsimd.dma_start(x2b[i][:], x2_ap[:, h0 : h0 + 8, :])

            emit_chunk_loads(0)
            for i in range(16):
                h0 = 8 * i
                if i + 1 < 16:
                    emit_chunk_loads(i + 1)
                xb = xb_tiles.pop(i)

                # QK: 8 transposes -> one bank -> one wide evac
                pst = transpose8(lambda j: xb[:, j * C : (j + 1) * C])
                xt = p_xT.tile([C, 1024], bf16, tag="xT")
                evac(xt[:], pst[:], "act" if i % 2 == 0 else "dve")
                for s2 in range(2):
                    pending.append(emit_qk_convs(xt, h0, 4 * s2))
                    drain(LAG)
                # V: 8 transposes -> one bank -> one wide evac
                pst = transpose8(
                    lambda j: x2b[i][:, j * C : (j + 1) * C]
                )
                xt = p_xT.tile([C, 1024], bf16, tag="xT")
                evac(xt[:], pst[:], "dve" if i % 2 == 0 else "act")
                for s2 in range(2):
                    pending.append(emit_v_convs(xt, h0, 4 * s2))
                    drain(LAG)
            drain(0)

            # ===== Phase A: attention over channels =====
            e_tiles = {}  # sg -> e tile [g, 4H] bf16, channels 4sg..4sg+3
            o_groups = [(c0, min(3, C - c0)) for c0 in range(0, C, 3)]
            next_og = 0

            def emit_o_group(c0, gs):
                pso = ps_a.tile([H, gs * 129], fp32, tag="ps")
                for j in range(gs):
                    c = c0 + j
                    et = e_tiles[c // 4]
                    nc.tensor.matmul(
                        pso[:, j * 129 : (j + 1) * 129],
                        et[:, (c % 4) * H : (c % 4 + 1) * H],
                        v_sb[:, c : c + W * C + 1 : C],
                        start=(j == 0), stop=(j == gs - 1),
                    )
                po = pso[:].rearrange("h (j x) -> h j x", x=129)
                rz = p_rz.tile([H, gs], fp32, tag="rz")
                nc.vector.reciprocal(rz[:], po[:, :, 128])
                if delta_zero:
                    # wide normalize: o = o_unnorm * (1/Z) with 1/Z
                    # broadcast along w; dst is o_sb pixel-major [h,(w,c)]
                    pox = pso[:].rearrange("h (j x) -> h x j", x=129)
                    rzb = rz[:].unsqueeze(1).broadcast_to([H, W, gs])
                    nc.vector.tensor_tensor(
                        o3[:, :, c0 : c0 + gs], pox[:, 0:W, :], rzb, OP.mult,
                    )
                else:
                    for j in range(gs):
                        c = c0 + j
                        dst = o3[:, :, c]
                        src_ap = po[:, j, 0:W]
                        if (c0 // 3) % 2 == 0:
                            nc.scalar.activation(
                                dst, src_ap, AF.Copy,
                                bias=float(delta[c]), scale=rz[:, j : j + 1],
                            )
                        else:
                            nc.vector.tensor_scalar(
                                dst, src_ap, rz[:, j : j + 1], float(delta[c]),
                                OP.mult, OP.add,
                            )

            for sg in range(32):  # 4-channel score groups
                pss = ps_a.tile([H, 4 * H], fp32, tag="ps")
                for j in range(4):
                    c = 4 * sg + j
                    nc.tensor.matmul(
                        pss[:, j * H : (j + 1) * H],
                        qk4[:, :, 1, c], qk4[:, :, 0, c],
                        start=(j == 0), stop=(j == 3),
                    )
                et = p_e.tile([H, 4 * H], bf16, tag="e4")
                nc.scalar.activation(et[:], pss[:], AF.Exp, scale=scale_val)
                e_tiles[sg] = et
                # drain o-groups whose channels are all exp'd already
                while (next_og < len(o_groups)
                       and o_groups[next_og][0] + o_groups[next_og][1] <= 4 * sg):
                    emit_o_group(*o_groups[next_og])
                    next_og += 1
            while next_og < len(o_groups):
                emit_o_group(*o_groups[next_og])
                next_og += 1

            # ===== Phase G: o -> oT -> conv -> sigmoid/BN/residual =====
            # Pipelined like QKV; x1 residual via SWDGE accumulate-DMA
            # (gpsimd Q7 is otherwise idle here).
            def emit_g_tail(xt, w8, half):
                def go():
                    w0 = w8 + 4 * half
                    psg = ps_a.tile([H, 512], fp32, tag="ps", name="psg")
                    for j in range(4):
                        nc.tensor.matmul(
                            psg[:, j * C : (j + 1) * C],
                            xt[:, (4 * half + j) * H : (4 * half + j + 1) * H],
                            ws[:], start=(j == 0), stop=(j == 3),
                        )
                    if bias_via_dve:
                        nc.vector.tensor_tensor(psg[:], psg[:], bsrep[:], OP.add)
                    g4 = p_g.tile([H, 512], bf16, tag="g4")
                    nc.scalar.activation(g4[:], psg[:], AF.Sigmoid)
                    nc.vector.tensor_tensor(g4[:], g4[:], arep[:], OP.mult)
                    if not b_zero:
                        nc.vector.tensor_tensor(g4[:], g4[:], brep[:], OP.add)
                    x2slice = x2b[w8 // 8][:, 4 * half * C : (4 * half + 4) * C]
                    t8 = t8_tiles[w8 // 8]
                    nc.vector.tensor_tensor(
                        t8[:, 4 * half * C : (4 * half + 4) * C],
                        x2slice, g4[:], OP.mult,
                    )
                    if half == 1:
                        # residual accumulate + store 8-wide (halves the
                        # SWDGE Q7 descriptor-gen cost per byte)
                        nc.gpsimd.dma_start(
                            t8[:], x1_ap[:, w8 : w8 + 8, :], accum_op=OP.add
                        )
                        nc.sync.dma_start(out_ap[:, w8 : w8 + 8, :], t8[:])
                return go

            t8_tiles = {}
            for w8 in range(0, W, 8):
                pst = transpose8(
                    lambda j: o_sb[:, (w8 + j) * C : (w8 + j + 1) * C]
                )
                xt = p_xT.tile([C, 1024], bf16, tag="xT")
                evac(xt[:], pst[:], "dve" if (w8 // 8) % 2 else "act")
                t8_tiles[w8 // 8] = p_out.tile([H, 1024], fp32, tag="t8", name="t8")
                for half in range(2):
                    pending.append(emit_g_tail(xt, w8, half))
                    drain(LAG)
            drain(0)

    nc.compile()
    return nc


def _prepare(inputs):
    """Host-side prep: derived small tensors + baked scalars."""
    x1 = np.ascontiguousarray(np.asarray(inputs["x1"], dtype=np.float32))
    x2 = np.ascontiguousarray(np.asarray(inputs["x2"], dtype=np.float32))
    Wq = np.asarray(inputs["Wq"], dtype=np.float32)
    Wk = np.asarray(inputs["Wk"], dtype=np.float32)
    Wv = np.asarray(inputs["Wv"], dtype=np.float32)
    Ws = np.asarray(inputs["Ws"], dtype=np.float32)
    bs = np.asarray(inputs["bs"], dtype=np.float32)
    scale = float(np.asarray(inputs["scale"]).reshape(-1)[0])
    gamma = np.asarray(inputs["gamma"], dtype=np.float32)
    beta = np.asarray(inputs["beta"], dtype=np.float32)
    mu = np.asarray(inputs["mu"], dtype=np.float32)
    var = np.asarray(inputs["var"], dtype=np.float32)

    a = gamma / np.sqrt(var + BN_EPS)
    b = beta - mu * a
    b_zero = bool(np.all(b == 0.0))

    # fold the sigmoid bias bs into o:  o' = o + delta with Ws^T delta = bs
    bias_via_dve = False
    delta = np.zeros(C, dtype=np.float64)
    if np.any(bs != 0.0):
        try:
            delta = np.linalg.solve(Ws.astype(np.float64).T, bs.astype(np.float64))
            resid = np.abs(Ws.T @ delta.astype(np.float32) - bs).max()
            if not np.isfinite(delta).all() or resid > 1e-5 * (1 + np.abs(bs).max()):
                raise np.linalg.LinAlgError("bad solve")
        except np.linalg.LinAlgError:
            delta = np.zeros(C, dtype=np.float64)
            bias_via_dve = True

    bf = ml_dtypes.bfloat16
    consts = {
        "wqk": np.concatenate([Wq, Wk], axis=1).astype(bf),
        "wv": Wv.astype(bf),
        "ws": Ws.astype(bf),
        "ident": np.eye(C, dtype=bf),
        "a_rep": np.tile(a, (C, 4)).astype(bf),
        "b_rep": np.tile(b, (C, 4)).astype(bf),
        "bs_rep": np.tile(bs, (C, 4)).astype(np.float32),
    }
    key = (scale, tuple(np.round(delta, 12)), bias_via_dve, b_zero)
    return x1, x2, consts, key, scale, delta, bias_via_dve, b_zero


def _get_nc(key, scale, delta, bias_via_dve, b_zero):
    if key not in _BUILD_CACHE:
        _BUILD_CACHE[key] = _build_program(scale, delta, bias_via_dve, b_zero)
    return _BUILD_CACHE[key]


def run(inputs, trace: bool = False):
    from concourse.bass_utils import run_bass_kernel_spmd

    x1, x2, consts, key, scale, delta, bias_via_dve, b_zero = _prepare(inputs)
    nc = _get_nc(key, scale, delta, bias_via_dve, b_zero)

    in_maps = []
    for core in range(N_CORES):
        m = dict(consts)
        m["x1"] = x1[core]
        m["x2"] = x2[core]
        in_maps.append(m)

    res = run_bass_kernel_spmd(
        nc, in_maps, core_ids=list(range(N_CORES)), trace=trace
    )
    out = np.stack([res.results[i]["out"] for i in range(N_CORES)], axis=0)
    return out.astype(np.float32), res


def kernel(**inputs) -> np.ndarray:
    out, _ = run(inputs, trace=False)
    return out
